# revision 13
# baseline (speedup 1.0000x reference)
# MiniGPT forward pass on 8 Trainium2 NeuronCores (Bass/Tile).
#
# Sharding: batch element b is handled by the core pair {2b, 2b+1}; each core
# owns 512 consecutive tokens of that sequence.  Everything except attention
# context is token-local, so the only cross-core traffic is one pairwise
# AllGather of (K, V) per layer.  Causality is enforced by per-core additive
# masks supplied as input data, so all 8 cores run one SPMD program.
#
# On-chip layout: activations are kept transposed [feature, token] so every
# projection is matmul(lhsT=W[c, f_tile], rhs=xT[c, t]) accumulating over
# 128-row c-tiles in fp32 PSUM.  Matmul inputs are bf16; the residual stream
# stays fp32 in SBUF.  Attention scores are computed transposed s^T[kj, qi]
# (this makes w @ v a plain matmul with no transposes); softmax skips the
# running-max (scores are bounded for this model's scale) and the denominator
# comes from a ones-column appended to V.

import os
import numpy as np
import ml_dtypes

B, T, C, H, HD, L, V = 4, 1024, 1024, 16, 64, 8, 32000
TL = 512          # tokens per core
P = 128
CT = C // P       # 8 c-tiles
FT = (4 * C) // P # 32 f-tiles
EPS = 1e-5
NCORES = 8
VT = 500          # lm_head column tile (64 * 500 = 32000)
NVT = V // VT
BF = ml_dtypes.bfloat16

L_BUILD = int(os.environ.get("KERNEL_LAYERS", str(L)))
RUN_LM = os.environ.get("KERNEL_SKIP_LM", "0") != "1"

_CACHE = {}


def _split_multiwaits(nc, mybir):
    """This container's walrus build crashes codegen on any instruction that
    carries more than one semaphore wait; hoist extras into standalone
    single-wait EventSemaphore instructions on the same engine."""
    for f in nc.m.functions:
        for blk in f.blocks:
            new_list = []
            for ins in blk.instructions:
                si = ins.sync_info
                if si is not None and si.on_wait is not None and len(si.on_wait) > 1:
                    waits = list(si.on_wait)
                    for k, w in enumerate(waits[:-1]):
                        new_list.append(mybir.InstEventSemaphore(
                            name=f"{ins.name}-splitw{k}", engine=ins.engine,
                            ins=[], outs=[],
                            sync_info=mybir.SyncInfo(on_wait=[w], on_update=[])))
                    ins.sync_info = mybir.SyncInfo(
                        on_wait=[waits[-1]], on_update=list(si.on_update or []))
                new_list.append(ins)
            blk.instructions[:] = new_list


def _build():
    import concourse.bass as bass
    import concourse.mybir as mybir
    import concourse.tile as tile

    bf16 = mybir.dt.bfloat16
    f32 = mybir.dt.float32
    AL = mybir.AluOpType
    AF = mybir.ActivationFunctionType

    nc = bass.Bass("TRN2", target_bir_lowering=False, debug=False,
                   num_devices=NCORES)

    x0T = nc.dram_tensor("x0T", [C, TL], f32, kind="ExternalInput")
    maskT = nc.dram_tensor("maskT", [T, TL], bf16, kind="ExternalInput")
    wqT = nc.dram_tensor("wqT", [L, C, C], bf16, kind="ExternalInput")
    wkT = nc.dram_tensor("wkT", [L, C, C], bf16, kind="ExternalInput")
    wvT = nc.dram_tensor("wvT", [L, C, C], bf16, kind="ExternalInput")
    wpT = nc.dram_tensor("wpT", [L, C, C], bf16, kind="ExternalInput")
    w1T = nc.dram_tensor("w1T", [L, C, 4 * C], bf16, kind="ExternalInput")
    w2T = nc.dram_tensor("w2T", [L, 4 * C, C], bf16, kind="ExternalInput")
    wlmT = nc.dram_tensor("wlmT", [C, V], bf16, kind="ExternalInput")
    g1d = nc.dram_tensor("g1d", [L, C], f32, kind="ExternalInput")
    be1d = nc.dram_tensor("be1d", [L, C], f32, kind="ExternalInput")
    g2d = nc.dram_tensor("g2d", [L, C], f32, kind="ExternalInput")
    be2d = nc.dram_tensor("be2d", [L, C], f32, kind="ExternalInput")
    gfd = nc.dram_tensor("gfd", [C], f32, kind="ExternalInput")
    befd = nc.dram_tensor("befd", [C], f32, kind="ExternalInput")
    bpd = nc.dram_tensor("bpd", [L, C], f32, kind="ExternalInput")
    b1d = nc.dram_tensor("b1d", [L, 4 * C], f32, kind="ExternalInput")
    b2d = nc.dram_tensor("b2d", [L, C], f32, kind="ExternalInput")
    out_logits = nc.dram_tensor("out_logits", [TL, V], f32,
                                kind="ExternalOutput")

    with tile.TileContext(nc) as tc:
        with tc.tile_pool(name="persist", bufs=1) as pe, \
             tc.tile_pool(name="mmps", bufs=3, space="PSUM") as mmps, \
             tc.tile_pool(name="ops", bufs=3, space="PSUM") as ops, \
             tc.tile_pool(name="stps", bufs=2, space="PSUM") as stps, \
             tc.tile_pool(name="wpool", bufs=4) as wpool, \
             tc.tile_pool(name="wbig", bufs=2) as wbig, \
             tc.tile_pool(name="f32t", bufs=4) as f32t, \
             tc.tile_pool(name="bc", bufs=4) as bc, \
             tc.tile_pool(name="xbsq", bufs=3) as xbsqp, \
             tc.tile_pool(name="sTp", bufs=3) as sTp, \
             tc.tile_pool(name="wTp", bufs=3) as wTp, \
             tc.tile_pool(name="stage", bufs=4) as stagep, \
             tc.tile_pool(name="rows", bufs=4) as rows, \
             tc.tile_pool(name="dram", bufs=1, space="DRAM") as dramp:

            # ---------- persistent SBUF state ----------
            xT = pe.tile([P, CT, TL], f32)          # residual stream
            hT = pe.tile([P, CT, TL], bf16)         # post-LN activations
            maskS = pe.tile([P, CT, TL], bf16)      # additive causal mask^T
            qT = pe.tile([P, CT, TL], bf16)
            kctx = pe.tile([P, CT, T], bf16)        # K^T for full context
            vctx = pe.tile([P, CT, H, HD + 1], bf16)  # V (+ones col), kj-tiled
            oT = pe.tile([P, CT, TL], bf16)
            ffT = pe.tile([P, FT, TL], bf16)
            ones_col = pe.tile([P, 1], bf16)        # stats lhsT
            ones_row = pe.tile([1, P], bf16)        # partition-bcast lhsT
            esel_lo = pe.tile([1, P], bf16)         # bcast selector, rows 0-63
            esel_hi = pe.tile([1, P], bf16)         # bcast selector, rows 64-127
            # LN / bias params, one [P, CT]-slice per layer
            g1S = pe.tile([P, CT, L], f32)
            be1S = pe.tile([P, CT, L], f32)
            g2S = pe.tile([P, CT, L], f32)
            be2S = pe.tile([P, CT, L], f32)
            bpS = pe.tile([P, CT, L], f32)
            b2S = pe.tile([P, CT, L], f32)
            b1S = pe.tile([P, FT, L], f32)
            gfS = pe.tile([P, CT], f32)
            befS = pe.tile([P, CT], f32)

            nc.vector.memset(ones_col[:], 1.0)
            nc.vector.memset(ones_row[:], 1.0)
            nc.vector.memset(esel_lo[:], 0.0)
            nc.vector.memset(esel_lo[0:1, 0:64], 1.0)
            nc.vector.memset(esel_hi[:], 0.0)
            nc.vector.memset(esel_hi[0:1, 64:P], 1.0)
            nc.vector.memset(vctx[:, :, :, HD:HD + 1], 1.0)
            eps_t = pe.tile([1, 1], f32)
            nc.vector.memset(eps_t[:], EPS)

            nc.sync.dma_start(xT[:], x0T.rearrange("(ct p) t -> p ct t", p=P))
            nc.sync.dma_start(maskS[:], maskT.rearrange("(ct p) t -> p ct t", p=P))
            for _l in range(L):
                for _t, _d in ((g1S, g1d), (be1S, be1d), (g2S, g2d),
                               (be2S, be2d), (bpS, bpd), (b2S, b2d)):
                    nc.sync.dma_start(
                        _t[:, :, _l],
                        _d[_l].rearrange("(ct p) -> p ct", p=P))
                nc.sync.dma_start(
                    b1S[:, :, _l],
                    b1d[_l].rearrange("(ft p) -> p ft", p=P))
            nc.sync.dma_start(gfS[:], gfd.rearrange("(ct p) -> p ct", p=P))
            nc.sync.dma_start(befS[:], befd.rearrange("(ct p) -> p ct", p=P))

            # collective bounce buffers: [K^T (C x TL) | V (TL x C) flattened]
            collin = dramp.tile([2 * C, TL], bf16)
            collout = dramp.tile([2, 2 * C, TL], bf16)

            def layernorm(gS, beS, li):
                """hT = LN(xT) * g + b, via matmul stats + K=1 bcast."""
                st1 = stps.tile([1, TL], f32, tag="st")
                st2 = stps.tile([1, TL], f32, tag="st")
                for i in range(CT):
                    xb = xbsqp.tile([P, 2, TL], bf16, tag="xbsq")
                    nc.scalar.copy(xb[:, 0, :], xT[:, i, :])
                    nc.scalar.square(xb[:, 1, :], xT[:, i, :])
                    nc.tensor.matmul(st1[:], ones_col[:], xb[:, 0, :],
                                     start=(i == 0), stop=(i == CT - 1))
                    nc.tensor.matmul(st2[:], ones_col[:], xb[:, 1, :],
                                     start=(i == 0), stop=(i == CT - 1))
                mean = rows.tile([1, TL], f32, tag="row")
                e2 = rows.tile([1, TL], f32, tag="row")
                var = rows.tile([1, TL], f32, tag="row")
                std = rows.tile([1, TL], f32, tag="row")
                rstd = rows.tile([1, TL], f32, tag="row")
                meanb = rows.tile([1, TL], bf16, tag="rowb")
                rstdb = rows.tile([1, TL], bf16, tag="rowb")
                nc.scalar.mul(mean[:], st1[:], 1.0 / C)
                nc.scalar.mul(e2[:], st2[:], 1.0 / C)
                # var = e2 - mean^2 ; std = sqrt(var + eps) ; rstd = 1/std
                m2 = rows.tile([1, TL], f32, tag="row")
                nc.vector.tensor_mul(m2[:], mean[:], mean[:])
                nc.vector.tensor_sub(var[:], e2[:], m2[:])
                nc.scalar.activation(std[:], var[:], AF.Sqrt, bias=eps_t[:, 0:1])
                nc.vector.reciprocal(rstd[:], std[:])
                nc.scalar.copy(meanb[:], mean[:])
                nc.scalar.copy(rstdb[:], rstd[:])
                bcM_ps = mmps.tile([P, TL], f32, tag="mm")
                bcR_ps = mmps.tile([P, TL], f32, tag="mm")
                nc.tensor.matmul(bcM_ps[:], ones_row[:], meanb[:])
                nc.tensor.matmul(bcR_ps[:], ones_row[:], rstdb[:])
                bcM = bc.tile([P, TL], f32, tag="bc")
                bcR = bc.tile([P, TL], f32, tag="bc")
                nc.vector.tensor_copy(bcM[:], bcM_ps[:])
                nc.vector.tensor_copy(bcR[:], bcR_ps[:])
                for i in range(CT):
                    t1 = f32t.tile([P, TL], f32, tag="f32t")
                    t2 = f32t.tile([P, TL], f32, tag="f32t")
                    nc.vector.tensor_sub(t1[:], xT[:, i, :], bcM[:])
                    nc.vector.tensor_mul(t2[:], t1[:], bcR[:])
                    nc.vector.scalar_tensor_tensor(
                        hT[:, i, :], t2[:], gS[:, i, li:li + 1],
                        beS[:, i, li:li + 1].to_broadcast((P, TL)),
                        op0=AL.mult, op1=AL.add)

            def load_wcol(dram3, li, i, n_sub, tag="w128"):
                """[*, 128*i : 128*(i+1)] column block as [P, n_sub, 128]."""
                t = wpool.tile([P, n_sub, P], bf16, tag=tag)
                nc.sync.dma_start(
                    t[:], dram3[li, :, P * i:P * (i + 1)]
                    .rearrange("(s p) m -> p s m", p=P))
                return t

            for li in range(L_BUILD):
                # ---------- LN1 ----------
                layernorm(g1S, be1S, li)

                # ---------- Q, K projections ----------
                for i in range(CT):
                    wq_t = load_wcol(wqT, li, i, CT)
                    ps = mmps.tile([P, TL], f32, tag="mm")
                    for ci in range(CT):
                        nc.tensor.matmul(ps[:], wq_t[:, ci, :], hT[:, ci, :],
                                         start=(ci == 0), stop=(ci == CT - 1))
                    nc.scalar.mul(qT[:, i, :], ps[:], float(HD) ** -0.5)
                for i in range(CT):
                    wk_t = load_wcol(wkT, li, i, CT)
                    ps = mmps.tile([P, TL], f32, tag="mm")
                    for ci in range(CT):
                        nc.tensor.matmul(ps[:], wk_t[:, ci, :], hT[:, ci, :],
                                         start=(ci == 0), stop=(ci == CT - 1))
                    kst = stagep.tile([P, TL], bf16, tag="stage")
                    nc.scalar.copy(kst[:], ps[:])
                    nc.sync.dma_start(
                        collin[0:C, :].rearrange("(ct p) t -> p ct t", p=P)[:, i, :],
                        kst[:])

                # ---------- V projection ([token, feature] layout) ----------
                for half in range(2):
                    wv_t = wbig.tile([P, CT, TL], bf16, tag="wbig")
                    nc.sync.dma_start(
                        wv_t[:], wvT[li, :, TL * half:TL * (half + 1)]
                        .rearrange("(ct p) m -> p ct m", p=P))
                    for tt in range(TL // P):
                        ps = mmps.tile([P, TL], f32, tag="mm")
                        for ci in range(CT):
                            nc.tensor.matmul(
                                ps[:], hT[:, ci, P * tt:P * (tt + 1)],
                                wv_t[:, ci, :],
                                start=(ci == 0), stop=(ci == CT - 1))
                        vst = stagep.tile([P, TL], bf16, tag="stage")
                        nc.scalar.copy(vst[:], ps[:])
                        # V flattened [TL, C] -> rows r = 2*t + half
                        nc.sync.dma_start(
                            collin[C:2 * C, :]
                            .rearrange("(t a) b -> t a b", a=2)
                            [P * tt:P * (tt + 1), half, :],
                            vst[:])

                # ---------- pairwise AllGather of (K^T | V) ----------
                nc.gpsimd.collective_compute(
                    "AllGather", mybir.AluOpType.bypass,
                    replica_groups=[[0, 1], [2, 3], [4, 5], [6, 7]],
                    ins=[collin.opt()], outs=[collout.opt()])
                for s in range(2):
                    nc.sync.dma_start(
                        kctx[:, :, TL * s:TL * (s + 1)],
                        collout[s, 0:C, :].rearrange("(ct p) t -> p ct t", p=P))
                    vsrc = collout[s, C:2 * C, :].rearrange(
                        "(jj p a) (hp d) -> p jj a hp d", jj=4, p=P, a=2, hp=8)
                    vdst = vctx[:, 4 * s:4 * (s + 1), :, 0:HD].rearrange(
                        "p jj (a hp) d -> p jj a hp d", a=2)
                    for jj in range(4):
                        nc.sync.dma_start(vdst[:, jj], vsrc[:, jj])

                # ---------- attention, head pairs ----------
                for hp in range(H // 2):
                    o_ps = [None, None]
                    recbs = [None, None]
                    for sub in range(2):
                        h = 2 * hp + sub
                        lo, hi = 64 * sub, 64 * sub + 64
                        o_ps[sub] = ops.tile([HD + 1, TL], f32, tag="o", name="o_ps")
                        for jj in range(CT):
                            s_ps = mmps.tile([P, TL], f32, tag="mm")
                            nc.tensor.matmul(
                                s_ps[:],
                                kctx[lo:hi, hp, P * jj:P * (jj + 1)],
                                qT[lo:hi, hp, :])
                            sT = sTp.tile([P, TL], bf16, tag="sT")
                            nc.vector.tensor_add(sT[:], s_ps[:],
                                                 maskS[:, jj, :])
                            wTt = wTp.tile([P, TL], bf16, tag="wT")
                            nc.scalar.activation(wTt[:], sT[:], AF.Exp)
                            nc.tensor.matmul(o_ps[sub][:], vctx[:, jj, h, :],
                                             wTt[:],
                                             start=(jj == 0),
                                             stop=(jj == CT - 1))
                        recf = rows.tile([1, TL], f32, tag="row", name="recf")
                        nc.vector.reciprocal(recf[:],
                                             o_ps[sub][HD:HD + 1, :])
                        recbs[sub] = rows.tile([1, TL], bf16, tag="rowb",
                                               name="recb")
                        nc.scalar.copy(recbs[sub][:], recf[:])
                    bcr_ps = mmps.tile([P, TL], f32, tag="mm")
                    nc.tensor.matmul(bcr_ps[:], esel_lo[:], recbs[0][:],
                                     start=True, stop=False)
                    nc.tensor.matmul(bcr_ps[:], esel_hi[:], recbs[1][:],
                                     start=False, stop=True)
                    bcr = bc.tile([P, TL], f32, tag="bc")
                    nc.vector.tensor_copy(bcr[:], bcr_ps[:])
                    for sub in range(2):
                        lo, hi = 64 * sub, 64 * sub + 64
                        nc.vector.tensor_mul(oT[lo:hi, hp, :],
                                             o_ps[sub][0:HD, :], bcr[lo:hi, :])

                # ---------- attention out projection + residual ----------
                for i in range(CT):
                    wp_t = load_wcol(wpT, li, i, CT)
                    ps = mmps.tile([P, TL], f32, tag="mm")
                    for ci in range(CT):
                        nc.tensor.matmul(ps[:], wp_t[:, ci, :], oT[:, ci, :],
                                         start=(ci == 0), stop=(ci == CT - 1))
                    nc.vector.scalar_tensor_tensor(
                        xT[:, i, :], ps[:], bpS[:, i, li:li + 1], xT[:, i, :],
                        op0=AL.add, op1=AL.add)

                # ---------- LN2 + FFN ----------
                layernorm(g2S, be2S, li)
                for fi in range(FT):
                    w1_t = load_wcol(w1T, li, fi, CT)
                    ps = mmps.tile([P, TL], f32, tag="mm")
                    for ci in range(CT):
                        nc.tensor.matmul(ps[:], w1_t[:, ci, :], hT[:, ci, :],
                                         start=(ci == 0), stop=(ci == CT - 1))
                    nc.scalar.activation(ffT[:, fi, :], ps[:], AF.Relu,
                                         bias=b1S[:, fi, li:li + 1])
                for i in range(CT):
                    w2_t = wbig.tile([P, FT, P], bf16, tag="wbig")
                    nc.sync.dma_start(
                        w2_t[:], w2T[li, :, P * i:P * (i + 1)]
                        .rearrange("(ft p) m -> p ft m", p=P))
                    ps = mmps.tile([P, TL], f32, tag="mm")
                    for fi in range(FT):
                        nc.tensor.matmul(ps[:], w2_t[:, fi, :], ffT[:, fi, :],
                                         start=(fi == 0), stop=(fi == FT - 1))
                    nc.vector.scalar_tensor_tensor(
                        xT[:, i, :], ps[:], b2S[:, i, li:li + 1], xT[:, i, :],
                        op0=AL.add, op1=AL.add)

            # ---------- final LN + lm_head ----------
            if RUN_LM:
                st1 = stps.tile([1, TL], f32, tag="st")
                st2 = stps.tile([1, TL], f32, tag="st")
                for i in range(CT):
                    xb = xbsqp.tile([P, 2, TL], bf16, tag="xbsq")
                    nc.scalar.copy(xb[:, 0, :], xT[:, i, :])
                    nc.scalar.square(xb[:, 1, :], xT[:, i, :])
                    nc.tensor.matmul(st1[:], ones_col[:], xb[:, 0, :],
                                     start=(i == 0), stop=(i == CT - 1))
                    nc.tensor.matmul(st2[:], ones_col[:], xb[:, 1, :],
                                     start=(i == 0), stop=(i == CT - 1))
                mean = rows.tile([1, TL], f32, tag="row")
                e2 = rows.tile([1, TL], f32, tag="row")
                var = rows.tile([1, TL], f32, tag="row")
                std = rows.tile([1, TL], f32, tag="row")
                rstd = rows.tile([1, TL], f32, tag="row")
                meanb = rows.tile([1, TL], bf16, tag="rowb")
                rstdb = rows.tile([1, TL], bf16, tag="rowb")
                nc.scalar.mul(mean[:], st1[:], 1.0 / C)
                nc.scalar.mul(e2[:], st2[:], 1.0 / C)
                m2 = rows.tile([1, TL], f32, tag="row")
                nc.vector.tensor_mul(m2[:], mean[:], mean[:])
                nc.vector.tensor_sub(var[:], e2[:], m2[:])
                nc.scalar.activation(std[:], var[:],
                                     mybir.ActivationFunctionType.Sqrt,
                                     bias=eps_t[:, 0:1])
                nc.vector.reciprocal(rstd[:], std[:])
                nc.scalar.copy(meanb[:], mean[:])
                nc.scalar.copy(rstdb[:], rstd[:])
                bcM_ps = mmps.tile([P, TL], f32, tag="mm")
                bcR_ps = mmps.tile([P, TL], f32, tag="mm")
                nc.tensor.matmul(bcM_ps[:], ones_row[:], meanb[:])
                nc.tensor.matmul(bcR_ps[:], ones_row[:], rstdb[:])
                bcM = bc.tile([P, TL], f32, tag="bc")
                bcR = bc.tile([P, TL], f32, tag="bc")
                nc.vector.tensor_copy(bcM[:], bcM_ps[:])
                nc.vector.tensor_copy(bcR[:], bcR_ps[:])
                for i in range(CT):
                    t1 = f32t.tile([P, TL], f32, tag="f32t")
                    t2 = f32t.tile([P, TL], f32, tag="f32t")
                    nc.vector.tensor_sub(t1[:], xT[:, i, :], bcM[:])
                    nc.vector.tensor_mul(t2[:], t1[:], bcR[:])
                    nc.vector.scalar_tensor_tensor(
                        hT[:, i, :], t2[:], gfS[:, i:i + 1],
                        befS[:, i:i + 1].to_broadcast((P, TL)),
                        op0=mybir.AluOpType.mult, op1=mybir.AluOpType.add)

                for vv in range(NVT):
                    wlm_t = wbig.tile([P, CT, TL], bf16, tag="wbig")
                    nc.sync.dma_start(
                        wlm_t[:, :, 0:VT],
                        wlmT[:, VT * vv:VT * (vv + 1)]
                        .rearrange("(ct p) m -> p ct m", p=P))
                    for tt in range(TL // P):
                        ps = mmps.tile([P, TL], f32, tag="mm")
                        for ci in range(CT):
                            nc.tensor.matmul(
                                ps[:, 0:VT], hT[:, ci, P * tt:P * (tt + 1)],
                                wlm_t[:, ci, 0:VT],
                                start=(ci == 0), stop=(ci == CT - 1))
                        lo = f32t.tile([P, TL], f32, tag="f32t")
                        if tt % 2 == 0:
                            nc.vector.tensor_copy(lo[:, 0:VT], ps[:, 0:VT])
                        else:
                            nc.scalar.copy(lo[:, 0:VT], ps[:, 0:VT])
                        nc.sync.dma_start(
                            out_logits[P * tt:P * (tt + 1),
                                       VT * vv:VT * (vv + 1)],
                            lo[:, 0:VT])
            else:
                # debug mode: emit the residual stream x [TL, C] into the
                # first C columns of out_logits
                for i in range(CT):
                    lo = f32t.tile([P, TL], f32, tag="f32t")
                    nc.vector.tensor_copy(lo[:], xT[:, i, :])
                    nc.sync.dma_start(
                        out_logits[0:TL, P * i:P * (i + 1)]
                        .rearrange("t p -> p t"),
                        lo[:])

    _split_multiwaits(nc, mybir)
    return nc


def _prep_inputs(idx, tok_emb, pos_emb, wq, wk, wv, wproj, bproj, ln1_g, ln1_b,
                 ln2_g, ln2_b, w1, b1, w2, b2, lnf_g, lnf_b, wlm, blm):
    f = np.float32
    x0 = np.asarray(tok_emb, f)[np.asarray(idx)] + np.asarray(pos_emb, f)[None, :T]
    shared = dict(
        wqT=np.ascontiguousarray(np.transpose(np.asarray(wq, f), (0, 2, 1, 3))
                                 .reshape(L, C, C)).astype(BF),
        wkT=np.ascontiguousarray(np.transpose(np.asarray(wk, f), (0, 2, 1, 3))
                                 .reshape(L, C, C)).astype(BF),
        wvT=np.ascontiguousarray(np.transpose(np.asarray(wv, f), (0, 2, 1, 3))
                                 .reshape(L, C, C)).astype(BF),
        wpT=np.ascontiguousarray(np.asarray(wproj, f)).astype(BF),
        w1T=np.ascontiguousarray(np.asarray(w1, f)).astype(BF),
        w2T=np.ascontiguousarray(np.asarray(w2, f)).astype(BF),
        wlmT=np.ascontiguousarray(np.asarray(wlm, f)).astype(BF),
        g1d=np.ascontiguousarray(np.asarray(ln1_g, f)),
        be1d=np.ascontiguousarray(np.asarray(ln1_b, f)),
        g2d=np.ascontiguousarray(np.asarray(ln2_g, f)),
        be2d=np.ascontiguousarray(np.asarray(ln2_b, f)),
        gfd=np.ascontiguousarray(np.asarray(lnf_g, f)),
        befd=np.ascontiguousarray(np.asarray(lnf_b, f)),
        bpd=np.ascontiguousarray(np.asarray(bproj, f)),
        b1d=np.ascontiguousarray(np.asarray(b1, f)),
        b2d=np.ascontiguousarray(np.asarray(b2, f)),
    )
    kj = np.arange(T)[:, None]
    in_maps = []
    for c in range(NCORES):
        b, s = c // 2, c % 2
        xs = np.ascontiguousarray(x0[b, TL * s:TL * (s + 1), :].T)
        qg = TL * s + np.arange(TL)[None, :]
        m = np.where(kj <= qg, 0.0, -1e9).astype(BF)
        in_maps.append(dict(shared, x0T=xs, maskT=np.ascontiguousarray(m)))
    return in_maps


def time_kernel(inputs, iters=4):
    """Measure on-device NEFF execution time: device-resident inputs, timed
    jitted shard_map executions (min over iters).  Mirrors
    bass2jax.run_bass_via_pjrt.  Not used by the grading path."""
    import time as _time
    import jax
    import concourse.mybir as mybir
    from concourse.bass2jax import (_bass_exec_p, install_neuronx_cc_hook,
                                    partition_id_tensor)
    from jax.experimental.shard_map import shard_map
    from jax.sharding import Mesh, NamedSharding, PartitionSpec

    if "nc" not in _CACHE:
        _CACHE["nc"] = _build()
    nc = _CACHE["nc"]
    install_neuronx_cc_hook()

    in_names, out_names, out_avals, zero_outs = [], [], [], []
    partition_name = (nc.partition_id_tensor.name
                      if nc.partition_id_tensor else None)
    for alloc in nc.m.functions[0].allocations:
        if not isinstance(alloc, mybir.MemoryLocationSet):
            continue
        name = alloc.memorylocations[0].name
        if alloc.kind == "ExternalInput":
            if name != partition_name:
                in_names.append(name)
        elif alloc.kind == "ExternalOutput":
            shape = tuple(alloc.tensor_shape)
            dtype = mybir.dt.np(alloc.dtype)
            out_names.append(name)
            out_avals.append(jax.core.ShapedArray(shape, dtype))
            zero_outs.append(np.zeros(shape, dtype))
    n_params = len(in_names)
    n_outs = len(out_avals)
    all_names = in_names + out_names
    if partition_name is not None:
        all_names.append(partition_name)
    donate = tuple(range(n_params, n_params + n_outs))

    def _body(*args):
        operands = list(args)
        if partition_name is not None:
            operands.append(partition_id_tensor())
        return tuple(_bass_exec_p.bind(
            *operands, out_avals=tuple(out_avals), in_names=tuple(all_names),
            out_names=tuple(out_names), lowering_input_output_aliases=(),
            sim_require_finite=True, sim_require_nnan=True, nc=nc))

    devices = jax.devices()[:NCORES]
    mesh = Mesh(np.asarray(devices), ("core",))
    spec = PartitionSpec("core")
    sharded = jax.jit(
        shard_map(_body, mesh=mesh, in_specs=(spec,) * (n_params + n_outs),
                  out_specs=(spec,) * n_outs, check_rep=False),
        donate_argnums=donate, keep_unused=True)

    in_maps = _prep_inputs(**inputs)
    concat_in = [np.concatenate([np.asarray(in_maps[c][k]) for c in
                                 range(NCORES)], axis=0) for k in in_names]
    sh = NamedSharding(mesh, spec)
    din = [jax.device_put(a, sh) for a in concat_in]
    jax.block_until_ready(din)
    times = []
    out = None
    for _ in range(iters + 1):
        dzeros = [jax.device_put(
            np.zeros((NCORES * z.shape[0], *z.shape[1:]), z.dtype), sh)
            for z in zero_outs]
        jax.block_until_ready(dzeros)
        t0 = _time.perf_counter()
        out = sharded(*din, *dzeros)
        jax.block_until_ready(out)
        times.append(_time.perf_counter() - t0)
    results = [
        {name: np.asarray(out[i]).reshape(NCORES, *out_avals[i].shape)[c]
         for i, name in enumerate(out_names)}
        for c in range(NCORES)]
    return min(times[1:]), times, results


def kernel(**inputs):
    from concourse.bass_utils import run_bass_kernel_spmd

    if "nc" not in _CACHE:
        _CACHE["nc"] = _build()
    nc = _CACHE["nc"]

    in_maps = _prep_inputs(**inputs)
    trace = os.environ.get("KERNEL_TRACE", "0") == "1"
    res = run_bass_kernel_spmd(nc, in_maps, core_ids=list(range(NCORES)),
                               trace=trace)
    _CACHE["last_result"] = res

    out = np.empty((B, T, V), np.float32)
    for c in range(NCORES):
        b, s = c // 2, c % 2
        out[b, TL * s:TL * (s + 1), :] = res.results[c]["out_logits"]
    out += np.asarray(inputs["blm"], np.float32)[None, None, :]
    return out


# revision 14
# speedup vs baseline: 1.1907x; 1.1907x over previous
# MiniGPT forward pass on 8 Trainium2 NeuronCores (Bass/Tile).
#
# Sharding: batch element b is handled by the core pair {2b, 2b+1}; each core
# owns 512 consecutive tokens of that sequence.  Everything except attention
# context is token-local, so the only cross-core traffic is one pairwise
# AllGather of (K, V) per layer.  Causality is enforced by per-core additive
# masks supplied as input data, so all 8 cores run one SPMD program.
#
# On-chip layout: activations are kept transposed [feature, token] so every
# projection is matmul(lhsT=W[c, f_tile], rhs=xT[c, t]) accumulating over
# 128-row c-tiles in fp32 PSUM.  Matmul inputs are bf16; the residual stream
# stays fp32 in SBUF.  Attention scores are computed transposed s^T[kj, qi]
# (this makes w @ v a plain matmul with no transposes); softmax skips the
# running-max (scores are bounded for this model's scale) and the denominator
# comes from a ones-column appended to V.

import os
import numpy as np
import ml_dtypes

B, T, C, H, HD, L, V = 4, 1024, 1024, 16, 64, 8, 32000
TL = 512          # tokens per core
P = 128
CT = C // P       # 8 c-tiles
FT = (4 * C) // P # 32 f-tiles
EPS = 1e-5
NCORES = 8
VT = 500          # lm_head column tile (64 * 500 = 32000)
NVT = V // VT
BF = ml_dtypes.bfloat16

L_BUILD = int(os.environ.get("KERNEL_LAYERS", str(L)))
RUN_LM = os.environ.get("KERNEL_SKIP_LM", "0") != "1"
NO_COLL = os.environ.get("KERNEL_NO_COLL", "0") == "1"

_CACHE = {}


def _split_multiwaits(nc, mybir):
    """This container's walrus build crashes codegen on any instruction that
    carries more than one semaphore wait; hoist extras into standalone
    single-wait EventSemaphore instructions on the same engine."""
    for f in nc.m.functions:
        for blk in f.blocks:
            new_list = []
            for ins in blk.instructions:
                si = ins.sync_info
                if si is not None and si.on_wait is not None and len(si.on_wait) > 1:
                    waits = list(si.on_wait)
                    for k, w in enumerate(waits[:-1]):
                        new_list.append(mybir.InstEventSemaphore(
                            name=f"{ins.name}-splitw{k}", engine=ins.engine,
                            ins=[], outs=[],
                            sync_info=mybir.SyncInfo(on_wait=[w], on_update=[])))
                    ins.sync_info = mybir.SyncInfo(
                        on_wait=[waits[-1]], on_update=list(si.on_update or []))
                new_list.append(ins)
            blk.instructions[:] = new_list


def _build():
    import concourse.bass as bass
    import concourse.mybir as mybir
    import concourse.tile as tile

    bf16 = mybir.dt.bfloat16
    f32 = mybir.dt.float32
    AL = mybir.AluOpType
    AF = mybir.ActivationFunctionType

    nc = bass.Bass("TRN2", target_bir_lowering=False, debug=False,
                   num_devices=NCORES)

    x0T = nc.dram_tensor("x0T", [C, TL], f32, kind="ExternalInput")
    maskT = nc.dram_tensor("maskT", [T, TL], bf16, kind="ExternalInput")
    wqT = nc.dram_tensor("wqT", [L, C, C], bf16, kind="ExternalInput")
    wkT = nc.dram_tensor("wkT", [L, C, C], bf16, kind="ExternalInput")
    wvT = nc.dram_tensor("wvT", [L, C, C], bf16, kind="ExternalInput")
    wpT = nc.dram_tensor("wpT", [L, C, C], bf16, kind="ExternalInput")
    w1T = nc.dram_tensor("w1T", [L, C, 4 * C], bf16, kind="ExternalInput")
    w2T = nc.dram_tensor("w2T", [L, 4 * C, C], bf16, kind="ExternalInput")
    wlmT = nc.dram_tensor("wlmT", [C, V], bf16, kind="ExternalInput")
    g1d = nc.dram_tensor("g1d", [L, C], f32, kind="ExternalInput")
    be1d = nc.dram_tensor("be1d", [L, C], f32, kind="ExternalInput")
    g2d = nc.dram_tensor("g2d", [L, C], f32, kind="ExternalInput")
    be2d = nc.dram_tensor("be2d", [L, C], f32, kind="ExternalInput")
    gfd = nc.dram_tensor("gfd", [C], f32, kind="ExternalInput")
    befd = nc.dram_tensor("befd", [C], f32, kind="ExternalInput")
    bpd = nc.dram_tensor("bpd", [L, C], f32, kind="ExternalInput")
    b1d = nc.dram_tensor("b1d", [L, 4 * C], f32, kind="ExternalInput")
    b2d = nc.dram_tensor("b2d", [L, C], f32, kind="ExternalInput")
    out_logits = nc.dram_tensor("out_logits", [TL, V], f32,
                                kind="ExternalOutput")

    with tile.TileContext(nc) as tc:
        with tc.tile_pool(name="persist", bufs=1) as pe, \
             tc.tile_pool(name="mmps", bufs=3, space="PSUM") as mmps, \
             tc.tile_pool(name="ops", bufs=3, space="PSUM") as ops, \
             tc.tile_pool(name="stps", bufs=2, space="PSUM") as stps, \
             tc.tile_pool(name="wpool", bufs=4) as wpool, \
             tc.tile_pool(name="wbig", bufs=2) as wbig, \
             tc.tile_pool(name="f32t", bufs=4) as f32t, \
             tc.tile_pool(name="bc", bufs=4) as bc, \
             tc.tile_pool(name="xbsq", bufs=3) as xbsqp, \
             tc.tile_pool(name="sTp", bufs=3) as sTp, \
             tc.tile_pool(name="wTp", bufs=3) as wTp, \
             tc.tile_pool(name="stage", bufs=4) as stagep, \
             tc.tile_pool(name="rows", bufs=4) as rows, \
             tc.tile_pool(name="dram", bufs=1, space="DRAM") as dramp:

            # ---------- persistent SBUF state ----------
            xT = pe.tile([P, CT, TL], f32)          # residual stream
            hT = pe.tile([P, CT, TL], bf16)         # post-LN activations
            maskS = pe.tile([P, CT, TL], bf16)      # additive causal mask^T
            qT = pe.tile([P, CT, TL], bf16)
            kctx = pe.tile([P, CT, T], bf16)        # K^T for full context
            vctx = pe.tile([P, CT, H, HD + 1], bf16)  # V (+ones col), kj-tiled
            oT = pe.tile([P, CT, TL], bf16)
            ffT = pe.tile([P, FT, TL], bf16)
            ones_col = pe.tile([P, 1], bf16)        # stats lhsT
            ones_row = pe.tile([1, P], bf16)        # partition-bcast lhsT
            esel_lo = pe.tile([1, P], bf16)         # bcast selector, rows 0-63
            esel_hi = pe.tile([1, P], bf16)         # bcast selector, rows 64-127
            # LN / bias params, one [P, CT]-slice per layer
            g1S = pe.tile([P, CT, L], f32)
            be1S = pe.tile([P, CT, L], f32)
            g2S = pe.tile([P, CT, L], f32)
            be2S = pe.tile([P, CT, L], f32)
            bpS = pe.tile([P, CT, L], f32)
            b2S = pe.tile([P, CT, L], f32)
            b1S = pe.tile([P, FT, L], f32)
            gfS = pe.tile([P, CT], f32)
            befS = pe.tile([P, CT], f32)

            nc.vector.memset(ones_col[:], 1.0)
            nc.vector.memset(ones_row[:], 1.0)
            nc.vector.memset(esel_lo[:], 0.0)
            nc.vector.memset(esel_lo[0:1, 0:64], 1.0)
            nc.vector.memset(esel_hi[:], 0.0)
            nc.vector.memset(esel_hi[0:1, 64:P], 1.0)
            nc.vector.memset(vctx[:, :, :, HD:HD + 1], 1.0)
            eps_t = pe.tile([1, 1], f32)
            nc.vector.memset(eps_t[:], EPS)

            nc.sync.dma_start(xT[:], x0T.rearrange("(ct p) t -> p ct t", p=P))
            nc.sync.dma_start(maskS[:], maskT.rearrange("(ct p) t -> p ct t", p=P))
            for _l in range(L):
                for _t, _d in ((g1S, g1d), (be1S, be1d), (g2S, g2d),
                               (be2S, be2d), (bpS, bpd), (b2S, b2d)):
                    nc.sync.dma_start(
                        _t[:, :, _l],
                        _d[_l].rearrange("(ct p) -> p ct", p=P))
                nc.sync.dma_start(
                    b1S[:, :, _l],
                    b1d[_l].rearrange("(ft p) -> p ft", p=P))
            nc.sync.dma_start(gfS[:], gfd.rearrange("(ct p) -> p ct", p=P))
            nc.sync.dma_start(befS[:], befd.rearrange("(ct p) -> p ct", p=P))

            # collective bounce buffers: [K^T (C x TL) | V (TL x C) flattened]
            collin = dramp.tile([2 * C, TL], bf16)
            collout = dramp.tile([2, 2 * C, TL], bf16)

            def layernorm(gS, beS, li):
                """hT = LN(xT) * g + b, via matmul stats + K=1 bcast."""
                st1 = stps.tile([1, TL], f32, tag="st")
                st2 = stps.tile([1, TL], f32, tag="st")
                for i in range(CT):
                    xb = xbsqp.tile([P, 2, TL], bf16, tag="xbsq")
                    nc.scalar.copy(xb[:, 0, :], xT[:, i, :])
                    nc.scalar.square(xb[:, 1, :], xT[:, i, :])
                    nc.tensor.matmul(st1[:], ones_col[:], xb[:, 0, :],
                                     start=(i == 0), stop=(i == CT - 1))
                    nc.tensor.matmul(st2[:], ones_col[:], xb[:, 1, :],
                                     start=(i == 0), stop=(i == CT - 1))
                mean = rows.tile([1, TL], f32, tag="row")
                e2 = rows.tile([1, TL], f32, tag="row")
                var = rows.tile([1, TL], f32, tag="row")
                std = rows.tile([1, TL], f32, tag="row")
                rstd = rows.tile([1, TL], f32, tag="row")
                meanb = rows.tile([1, TL], bf16, tag="rowb")
                rstdb = rows.tile([1, TL], bf16, tag="rowb")
                nc.scalar.mul(mean[:], st1[:], 1.0 / C)
                nc.scalar.mul(e2[:], st2[:], 1.0 / C)
                # var = e2 - mean^2 ; std = sqrt(var + eps) ; rstd = 1/std
                m2 = rows.tile([1, TL], f32, tag="row")
                nc.vector.tensor_mul(m2[:], mean[:], mean[:])
                nc.vector.tensor_sub(var[:], e2[:], m2[:])
                nc.scalar.activation(std[:], var[:], AF.Sqrt, bias=eps_t[:, 0:1])
                nc.vector.reciprocal(rstd[:], std[:])
                nc.scalar.copy(meanb[:], mean[:])
                nc.scalar.copy(rstdb[:], rstd[:])
                bcM_ps = mmps.tile([P, TL], f32, tag="mm")
                bcR_ps = mmps.tile([P, TL], f32, tag="mm")
                nc.tensor.matmul(bcM_ps[:], ones_row[:], meanb[:])
                nc.tensor.matmul(bcR_ps[:], ones_row[:], rstdb[:])
                bcM = bc.tile([P, TL], f32, tag="bc")
                bcR = bc.tile([P, TL], f32, tag="bc")
                nc.vector.tensor_copy(bcM[:], bcM_ps[:])
                nc.vector.tensor_copy(bcR[:], bcR_ps[:])
                for i in range(CT):
                    t1 = f32t.tile([P, TL], f32, tag="f32t")
                    t2 = f32t.tile([P, TL], f32, tag="f32t")
                    nc.vector.tensor_sub(t1[:], xT[:, i, :], bcM[:])
                    nc.vector.tensor_mul(t2[:], t1[:], bcR[:])
                    nc.vector.scalar_tensor_tensor(
                        hT[:, i, :], t2[:], gS[:, i, li:li + 1],
                        beS[:, i, li:li + 1].to_broadcast((P, TL)),
                        op0=AL.mult, op1=AL.add)

            def load_wcol(dram3, li, i, n_sub, tag="w128"):
                """[*, 128*i : 128*(i+1)] column block as [P, n_sub, 128]."""
                t = wpool.tile([P, n_sub, P], bf16, tag=tag)
                nc.sync.dma_start(
                    t[:], dram3[li, :, P * i:P * (i + 1)]
                    .rearrange("(s p) m -> p s m", p=P))
                return t

            for li in range(L_BUILD):
                # ---------- LN1 ----------
                layernorm(g1S, be1S, li)

                # ---------- Q, K projections ----------
                for i in range(CT):
                    wq_t = load_wcol(wqT, li, i, CT)
                    ps = mmps.tile([P, TL], f32, tag="mm")
                    for ci in range(CT):
                        nc.tensor.matmul(ps[:], wq_t[:, ci, :], hT[:, ci, :],
                                         start=(ci == 0), stop=(ci == CT - 1))
                    nc.scalar.mul(qT[:, i, :], ps[:], float(HD) ** -0.5)
                for i in range(CT):
                    wk_t = load_wcol(wkT, li, i, CT)
                    ps = mmps.tile([P, TL], f32, tag="mm")
                    for ci in range(CT):
                        nc.tensor.matmul(ps[:], wk_t[:, ci, :], hT[:, ci, :],
                                         start=(ci == 0), stop=(ci == CT - 1))
                    kst = stagep.tile([P, TL], bf16, tag="stage")
                    nc.scalar.copy(kst[:], ps[:])
                    nc.sync.dma_start(
                        collin[0:C, :].rearrange("(ct p) t -> p ct t", p=P)[:, i, :],
                        kst[:])

                # ---------- V projection ([token, feature] layout) ----------
                for half in range(2):
                    wv_t = wbig.tile([P, CT, TL], bf16, tag="wbig")
                    nc.sync.dma_start(
                        wv_t[:], wvT[li, :, TL * half:TL * (half + 1)]
                        .rearrange("(ct p) m -> p ct m", p=P))
                    for tt in range(TL // P):
                        ps = mmps.tile([P, TL], f32, tag="mm")
                        for ci in range(CT):
                            nc.tensor.matmul(
                                ps[:], hT[:, ci, P * tt:P * (tt + 1)],
                                wv_t[:, ci, :],
                                start=(ci == 0), stop=(ci == CT - 1))
                        vst = stagep.tile([P, TL], bf16, tag="stage")
                        nc.scalar.copy(vst[:], ps[:])
                        # V flattened [TL, C] -> rows r = 2*t + half
                        nc.sync.dma_start(
                            collin[C:2 * C, :]
                            .rearrange("(t a) b -> t a b", a=2)
                            [P * tt:P * (tt + 1), half, :],
                            vst[:])

                # ---------- pairwise AllGather of (K^T | V) ----------
                if not NO_COLL:
                    nc.gpsimd.collective_compute(
                        "AllGather", mybir.AluOpType.bypass,
                        replica_groups=[[0, 1], [2, 3], [4, 5], [6, 7]],
                        ins=[collin.opt()], outs=[collout.opt()])
                for s in range(2):
                    csrc = collin if NO_COLL else collout[s]
                    nc.sync.dma_start(
                        kctx[:, :, TL * s:TL * (s + 1)],
                        csrc[0:C, :].rearrange("(ct p) t -> p ct t", p=P))
                    vsrc = csrc[C:2 * C, :].rearrange(
                        "(jj p a) (hp d) -> p jj a hp d", jj=4, p=P, a=2, hp=8)
                    vdst = vctx[:, 4 * s:4 * (s + 1), :, 0:HD].rearrange(
                        "p jj (a hp) d -> p jj a hp d", a=2)
                    for jj in range(4):
                        nc.sync.dma_start(vdst[:, jj], vsrc[:, jj])

                # ---------- attention, head pairs ----------
                for hp in range(H // 2):
                    o_ps = [None, None]
                    recbs = [None, None]
                    for sub in range(2):
                        h = 2 * hp + sub
                        lo, hi = 64 * sub, 64 * sub + 64
                        o_ps[sub] = ops.tile([HD + 1, TL], f32, tag="o", name="o_ps")
                        for jj in range(CT):
                            s_ps = mmps.tile([P, TL], f32, tag="mm")
                            nc.tensor.matmul(
                                s_ps[:],
                                kctx[lo:hi, hp, P * jj:P * (jj + 1)],
                                qT[lo:hi, hp, :])
                            sT = sTp.tile([P, TL], bf16, tag="sT")
                            nc.vector.tensor_add(sT[:], s_ps[:],
                                                 maskS[:, jj, :])
                            wTt = wTp.tile([P, TL], bf16, tag="wT")
                            nc.scalar.activation(wTt[:], sT[:], AF.Exp)
                            nc.tensor.matmul(o_ps[sub][:], vctx[:, jj, h, :],
                                             wTt[:],
                                             start=(jj == 0),
                                             stop=(jj == CT - 1))
                        recf = rows.tile([1, TL], f32, tag="row", name="recf")
                        nc.vector.reciprocal(recf[:],
                                             o_ps[sub][HD:HD + 1, :])
                        recbs[sub] = rows.tile([1, TL], bf16, tag="rowb",
                                               name="recb")
                        nc.scalar.copy(recbs[sub][:], recf[:])
                    bcr_ps = mmps.tile([P, TL], f32, tag="mm")
                    nc.tensor.matmul(bcr_ps[:], esel_lo[:], recbs[0][:],
                                     start=True, stop=False)
                    nc.tensor.matmul(bcr_ps[:], esel_hi[:], recbs[1][:],
                                     start=False, stop=True)
                    bcr = bc.tile([P, TL], f32, tag="bc")
                    nc.vector.tensor_copy(bcr[:], bcr_ps[:])
                    for sub in range(2):
                        lo, hi = 64 * sub, 64 * sub + 64
                        nc.vector.tensor_mul(oT[lo:hi, hp, :],
                                             o_ps[sub][0:HD, :], bcr[lo:hi, :])

                # ---------- attention out projection + residual ----------
                for i in range(CT):
                    wp_t = load_wcol(wpT, li, i, CT)
                    ps = mmps.tile([P, TL], f32, tag="mm")
                    for ci in range(CT):
                        nc.tensor.matmul(ps[:], wp_t[:, ci, :], oT[:, ci, :],
                                         start=(ci == 0), stop=(ci == CT - 1))
                    nc.vector.scalar_tensor_tensor(
                        xT[:, i, :], ps[:], bpS[:, i, li:li + 1], xT[:, i, :],
                        op0=AL.add, op1=AL.add)

                # ---------- LN2 + FFN ----------
                layernorm(g2S, be2S, li)
                for fi in range(FT):
                    w1_t = load_wcol(w1T, li, fi, CT)
                    ps = mmps.tile([P, TL], f32, tag="mm")
                    for ci in range(CT):
                        nc.tensor.matmul(ps[:], w1_t[:, ci, :], hT[:, ci, :],
                                         start=(ci == 0), stop=(ci == CT - 1))
                    nc.scalar.activation(ffT[:, fi, :], ps[:], AF.Relu,
                                         bias=b1S[:, fi, li:li + 1])
                for i in range(CT):
                    w2_t = wbig.tile([P, FT, P], bf16, tag="wbig")
                    nc.sync.dma_start(
                        w2_t[:], w2T[li, :, P * i:P * (i + 1)]
                        .rearrange("(ft p) m -> p ft m", p=P))
                    ps = mmps.tile([P, TL], f32, tag="mm")
                    for fi in range(FT):
                        nc.tensor.matmul(ps[:], w2_t[:, fi, :], ffT[:, fi, :],
                                         start=(fi == 0), stop=(fi == FT - 1))
                    nc.vector.scalar_tensor_tensor(
                        xT[:, i, :], ps[:], b2S[:, i, li:li + 1], xT[:, i, :],
                        op0=AL.add, op1=AL.add)

            # ---------- final LN + lm_head ----------
            if RUN_LM:
                st1 = stps.tile([1, TL], f32, tag="st")
                st2 = stps.tile([1, TL], f32, tag="st")
                for i in range(CT):
                    xb = xbsqp.tile([P, 2, TL], bf16, tag="xbsq")
                    nc.scalar.copy(xb[:, 0, :], xT[:, i, :])
                    nc.scalar.square(xb[:, 1, :], xT[:, i, :])
                    nc.tensor.matmul(st1[:], ones_col[:], xb[:, 0, :],
                                     start=(i == 0), stop=(i == CT - 1))
                    nc.tensor.matmul(st2[:], ones_col[:], xb[:, 1, :],
                                     start=(i == 0), stop=(i == CT - 1))
                mean = rows.tile([1, TL], f32, tag="row")
                e2 = rows.tile([1, TL], f32, tag="row")
                var = rows.tile([1, TL], f32, tag="row")
                std = rows.tile([1, TL], f32, tag="row")
                rstd = rows.tile([1, TL], f32, tag="row")
                meanb = rows.tile([1, TL], bf16, tag="rowb")
                rstdb = rows.tile([1, TL], bf16, tag="rowb")
                nc.scalar.mul(mean[:], st1[:], 1.0 / C)
                nc.scalar.mul(e2[:], st2[:], 1.0 / C)
                m2 = rows.tile([1, TL], f32, tag="row")
                nc.vector.tensor_mul(m2[:], mean[:], mean[:])
                nc.vector.tensor_sub(var[:], e2[:], m2[:])
                nc.scalar.activation(std[:], var[:],
                                     mybir.ActivationFunctionType.Sqrt,
                                     bias=eps_t[:, 0:1])
                nc.vector.reciprocal(rstd[:], std[:])
                nc.scalar.copy(meanb[:], mean[:])
                nc.scalar.copy(rstdb[:], rstd[:])
                bcM_ps = mmps.tile([P, TL], f32, tag="mm")
                bcR_ps = mmps.tile([P, TL], f32, tag="mm")
                nc.tensor.matmul(bcM_ps[:], ones_row[:], meanb[:])
                nc.tensor.matmul(bcR_ps[:], ones_row[:], rstdb[:])
                bcM = bc.tile([P, TL], f32, tag="bc")
                bcR = bc.tile([P, TL], f32, tag="bc")
                nc.vector.tensor_copy(bcM[:], bcM_ps[:])
                nc.vector.tensor_copy(bcR[:], bcR_ps[:])
                for i in range(CT):
                    t1 = f32t.tile([P, TL], f32, tag="f32t")
                    t2 = f32t.tile([P, TL], f32, tag="f32t")
                    nc.vector.tensor_sub(t1[:], xT[:, i, :], bcM[:])
                    nc.vector.tensor_mul(t2[:], t1[:], bcR[:])
                    nc.vector.scalar_tensor_tensor(
                        hT[:, i, :], t2[:], gfS[:, i:i + 1],
                        befS[:, i:i + 1].to_broadcast((P, TL)),
                        op0=mybir.AluOpType.mult, op1=mybir.AluOpType.add)

                for vv in range(NVT):
                    wlm_t = wbig.tile([P, CT, TL], bf16, tag="wbig")
                    nc.sync.dma_start(
                        wlm_t[:, :, 0:VT],
                        wlmT[:, VT * vv:VT * (vv + 1)]
                        .rearrange("(ct p) m -> p ct m", p=P))
                    for tt in range(TL // P):
                        ps = mmps.tile([P, TL], f32, tag="mm")
                        for ci in range(CT):
                            nc.tensor.matmul(
                                ps[:, 0:VT], hT[:, ci, P * tt:P * (tt + 1)],
                                wlm_t[:, ci, 0:VT],
                                start=(ci == 0), stop=(ci == CT - 1))
                        lo = f32t.tile([P, TL], f32, tag="f32t")
                        if tt % 2 == 0:
                            nc.vector.tensor_copy(lo[:, 0:VT], ps[:, 0:VT])
                        else:
                            nc.scalar.copy(lo[:, 0:VT], ps[:, 0:VT])
                        nc.sync.dma_start(
                            out_logits[P * tt:P * (tt + 1),
                                       VT * vv:VT * (vv + 1)],
                            lo[:, 0:VT])
            else:
                # debug mode: emit the residual stream x [TL, C] into the
                # first C columns of out_logits
                for i in range(CT):
                    lo = f32t.tile([P, TL], f32, tag="f32t")
                    nc.vector.tensor_copy(lo[:], xT[:, i, :])
                    nc.sync.dma_start(
                        out_logits[0:TL, P * i:P * (i + 1)]
                        .rearrange("t p -> p t"),
                        lo[:])

    _split_multiwaits(nc, mybir)
    return nc


def _prep_inputs(idx, tok_emb, pos_emb, wq, wk, wv, wproj, bproj, ln1_g, ln1_b,
                 ln2_g, ln2_b, w1, b1, w2, b2, lnf_g, lnf_b, wlm, blm):
    f = np.float32
    x0 = np.asarray(tok_emb, f)[np.asarray(idx)] + np.asarray(pos_emb, f)[None, :T]
    shared = dict(
        wqT=np.ascontiguousarray(np.transpose(np.asarray(wq, f), (0, 2, 1, 3))
                                 .reshape(L, C, C)).astype(BF),
        wkT=np.ascontiguousarray(np.transpose(np.asarray(wk, f), (0, 2, 1, 3))
                                 .reshape(L, C, C)).astype(BF),
        wvT=np.ascontiguousarray(np.transpose(np.asarray(wv, f), (0, 2, 1, 3))
                                 .reshape(L, C, C)).astype(BF),
        wpT=np.ascontiguousarray(np.asarray(wproj, f)).astype(BF),
        w1T=np.ascontiguousarray(np.asarray(w1, f)).astype(BF),
        w2T=np.ascontiguousarray(np.asarray(w2, f)).astype(BF),
        wlmT=np.ascontiguousarray(np.asarray(wlm, f)).astype(BF),
        g1d=np.ascontiguousarray(np.asarray(ln1_g, f)),
        be1d=np.ascontiguousarray(np.asarray(ln1_b, f)),
        g2d=np.ascontiguousarray(np.asarray(ln2_g, f)),
        be2d=np.ascontiguousarray(np.asarray(ln2_b, f)),
        gfd=np.ascontiguousarray(np.asarray(lnf_g, f)),
        befd=np.ascontiguousarray(np.asarray(lnf_b, f)),
        bpd=np.ascontiguousarray(np.asarray(bproj, f)),
        b1d=np.ascontiguousarray(np.asarray(b1, f)),
        b2d=np.ascontiguousarray(np.asarray(b2, f)),
    )
    kj = np.arange(T)[:, None]
    in_maps = []
    for c in range(NCORES):
        b, s = c // 2, c % 2
        xs = np.ascontiguousarray(x0[b, TL * s:TL * (s + 1), :].T)
        qg = TL * s + np.arange(TL)[None, :]
        m = np.where(kj <= qg, 0.0, -1e9).astype(BF)
        in_maps.append(dict(shared, x0T=xs, maskT=np.ascontiguousarray(m)))
    return in_maps


def time_kernel(inputs, iters=4):
    """Measure on-device NEFF execution time: device-resident inputs, timed
    jitted shard_map executions (min over iters).  Mirrors
    bass2jax.run_bass_via_pjrt.  Not used by the grading path."""
    import time as _time
    import jax
    import concourse.mybir as mybir
    from concourse.bass2jax import (_bass_exec_p, install_neuronx_cc_hook,
                                    partition_id_tensor)
    from jax.experimental.shard_map import shard_map
    from jax.sharding import Mesh, NamedSharding, PartitionSpec

    if "nc" not in _CACHE:
        _CACHE["nc"] = _build()
    nc = _CACHE["nc"]
    install_neuronx_cc_hook()

    in_names, out_names, out_avals, zero_outs = [], [], [], []
    partition_name = (nc.partition_id_tensor.name
                      if nc.partition_id_tensor else None)
    for alloc in nc.m.functions[0].allocations:
        if not isinstance(alloc, mybir.MemoryLocationSet):
            continue
        name = alloc.memorylocations[0].name
        if alloc.kind == "ExternalInput":
            if name != partition_name:
                in_names.append(name)
        elif alloc.kind == "ExternalOutput":
            shape = tuple(alloc.tensor_shape)
            dtype = mybir.dt.np(alloc.dtype)
            out_names.append(name)
            out_avals.append(jax.core.ShapedArray(shape, dtype))
            zero_outs.append(np.zeros(shape, dtype))
    n_params = len(in_names)
    n_outs = len(out_avals)
    all_names = in_names + out_names
    if partition_name is not None:
        all_names.append(partition_name)
    donate = tuple(range(n_params, n_params + n_outs))

    def _body(*args):
        operands = list(args)
        if partition_name is not None:
            operands.append(partition_id_tensor())
        return tuple(_bass_exec_p.bind(
            *operands, out_avals=tuple(out_avals), in_names=tuple(all_names),
            out_names=tuple(out_names), lowering_input_output_aliases=(),
            sim_require_finite=True, sim_require_nnan=True, nc=nc))

    devices = jax.devices()[:NCORES]
    mesh = Mesh(np.asarray(devices), ("core",))
    spec = PartitionSpec("core")
    sharded = jax.jit(
        shard_map(_body, mesh=mesh, in_specs=(spec,) * (n_params + n_outs),
                  out_specs=(spec,) * n_outs, check_rep=False),
        donate_argnums=donate, keep_unused=True)

    in_maps = _prep_inputs(**inputs)
    concat_in = [np.concatenate([np.asarray(in_maps[c][k]) for c in
                                 range(NCORES)], axis=0) for k in in_names]
    sh = NamedSharding(mesh, spec)
    din = [jax.device_put(a, sh) for a in concat_in]
    jax.block_until_ready(din)
    times = []
    out = None
    for _ in range(iters + 1):
        dzeros = [jax.device_put(
            np.zeros((NCORES * z.shape[0], *z.shape[1:]), z.dtype), sh)
            for z in zero_outs]
        jax.block_until_ready(dzeros)
        t0 = _time.perf_counter()
        out = sharded(*din, *dzeros)
        jax.block_until_ready(out)
        times.append(_time.perf_counter() - t0)
    results = [
        {name: np.asarray(out[i]).reshape(NCORES, *out_avals[i].shape)[c]
         for i, name in enumerate(out_names)}
        for c in range(NCORES)]
    return min(times[1:]), times, results


def kernel(**inputs):
    from concourse.bass_utils import run_bass_kernel_spmd

    if "nc" not in _CACHE:
        _CACHE["nc"] = _build()
    nc = _CACHE["nc"]

    in_maps = _prep_inputs(**inputs)
    trace = os.environ.get("KERNEL_TRACE", "0") == "1"
    res = run_bass_kernel_spmd(nc, in_maps, core_ids=list(range(NCORES)),
                               trace=trace)
    _CACHE["last_result"] = res

    out = np.empty((B, T, V), np.float32)
    for c in range(NCORES):
        b, s = c // 2, c % 2
        out[b, TL * s:TL * (s + 1), :] = res.results[c]["out_logits"]
    out += np.asarray(inputs["blm"], np.float32)[None, None, :]
    return out
